# revision 5
# baseline (speedup 1.0000x reference)
"""ExpanderGIN message-passing kernel for 8 Trainium2 NeuronCores (v4).

out = relu((x + segment_sum(x[src], dst)) @ W.T + b)

v7 = v5/v6 + on-chip index replication (uploads the unique 16-partition
index array and replicates 8x on DVE, moving ~1.1MB/core off the
bottleneck DMA engines) + deeper pipeline buffers.

v5 = v4 + src->quarter balancing.  Source nodes are greedily assigned
to the 4 quarter-tables to equalize per-(core,tile,quarter) edge counts,
which tightens group packing to ~5 tiles/group at ~1000 of the 1024 max
indices per gather instruction (less padding, fewer instructions).

v4 vs v21: group-level slot packing.  Edge slots are 128-aligned only per
(gather-group, quarter) instead of per (tile, quarter), cutting gather
padding from ~28%% to ~6%% and letting every gather instruction carry a
full 1024 indices (~20-24 SWDGE instructions/core instead of ~104).
Chunks of 128 edges may now span several destination tiles; each
(chunk, tile) incidence gets its own one-hot column whose values are
slot-position-relative-to-that-tile (out-of-tile edges fall outside
[0,127] and compare to zero), so TensorE accumulates exactly the right
rows into each tile's PSUM.  The incidence set is the union over all 8
cores (the SPMD program is shared; per-core columns zero out foreign
incidences).  bf16 data path throughout; bf16 output upcast on host.
"""

import numpy as np
import ml_dtypes

BF16 = ml_dtypes.bfloat16

N = 100000
E = 625000
D = 128
NC = 8            # cores
NPC = N // NC     # 12500 nodes per core
P = 128
TPC = (NPC + P - 1) // P   # 98 tiles per core
SLOTS = TPC * P            # 12544 slots per core
NQ = 4                     # quarter tables (int16 index limit)
QROWS = N // NQ            # 25000
MAXI = 1024                # dma_gather num_idxs hard limit

_f32 = np.float32
SENT = 30000.0             # sentinel dst value for pad slots


def _ceil128(v):
    return (int(v) + P - 1) // P * P


def _preprocess(edge_index):
    src = np.asarray(edge_index[0]).astype(np.int64)
    dst = np.asarray(edge_index[1]).astype(np.int64)
    deg = np.bincount(dst, minlength=N)

    # serpentine degree-balanced node -> slot assignment per core
    node_of = np.full((NC, SLOTS), -1, np.int64)
    slot_of = np.empty(N, np.int64)
    for c in range(NC):
        nodes = np.arange(c * NPC, (c + 1) * NPC)
        order = nodes[np.argsort(-deg[nodes], kind="stable")]
        padded = np.concatenate([order, np.full(SLOTS - NPC, -1, np.int64)])
        arr = padded.reshape(P, TPC).copy()
        arr[1::2] = arr[1::2, ::-1]
        node_of[c] = arr.T.reshape(-1)
        m = node_of[c] >= 0
        slot_of[node_of[c][m]] = np.nonzero(m)[0]

    ec = dst // NPC
    eslot = slot_of[dst]
    et = eslot // P
    epos = (eslot % P).astype(np.int64)

    # greedy src->quarter assignment balancing cnt[(core,tile), q]
    ekey = (ec * TPC + et).astype(np.int64)
    so = np.argsort(src, kind="stable")
    s_sorted = src[so]
    k_sorted = ekey[so]
    starts = np.searchsorted(s_sorted, np.arange(N + 1))
    deg_src = starts[1:] - starts[:-1]
    order_nodes = np.argsort(-deg_src, kind="stable")
    cntq = np.zeros((NC * TPC, NQ), np.int32)
    cap = np.full(NQ, QROWS, np.int64)
    qof = np.empty(N, np.int8)
    for s in order_nodes:
        a, b2 = int(starts[s]), int(starts[s + 1])
        if a == b2:
            q = int(np.argmax(cap))
        else:
            ks = k_sorted[a:b2]
            sc = cntq[ks].sum(axis=0)
            for q in np.argsort(sc, kind="stable"):
                q = int(q)
                if cap[q] > 0:
                    break
        qof[s] = q
        cap[q] -= 1
        if a != b2:
            np.add.at(cntq, (ks, q), 1)
    assert (cap >= 0).all()

    # table layout: quarter q rows = its nodes in ascending id order
    rank_of_node = np.empty(N, np.int64)
    xtab_order = np.zeros(NQ * QROWS, np.int64)
    for q in range(NQ):
        nodes_q = np.nonzero(qof == q)[0]
        rank_of_node[nodes_q] = np.arange(len(nodes_q))
        xtab_order[q * QROWS : q * QROWS + len(nodes_q)] = nodes_q
    eq = qof[src].astype(np.int64)
    eqidx = rank_of_node[src].astype(np.int16)

    cnt = np.bincount(
        (ec * TPC + et) * NQ + eq, minlength=NC * TPC * NQ
    ).reshape(NC, TPC, NQ)

    # pack tiles into groups: per quarter, ceil128(max-core count) <= MAXI
    groups = []
    t0 = 0
    while t0 < TPC:
        t1 = t0 + 1
        while t1 < TPC and all(
            _ceil128(cnt[:, t0 : t1 + 1, q].sum(axis=1).max()) <= MAXI
            for q in range(NQ)
        ):
            t1 += 1
        groups.append((t0, t1))
        t0 = t1
    NG = len(groups)
    gid_of_t = np.empty(TPC, np.int64)
    for g, (ta, tb) in enumerate(groups):
        gid_of_t[ta:tb] = g

    L = np.zeros((NG, NQ), np.int64)       # padded region length per (g,q)
    for g, (ta, tb) in enumerate(groups):
        for q in range(NQ):
            L[g, q] = _ceil128(cnt[:, ta:tb, q].sum(axis=1).max())
            assert L[g, q] <= MAXI
    slot_start = np.zeros((NG, NQ), np.int64)
    pos = 0
    for g in range(NG):
        for q in range(NQ):
            slot_start[g, q] = pos
            pos += L[g, q]
    S_total = pos
    assert S_total % P == 0

    # rank of each edge within (core, group, quarter), tiles ascending
    eg = gid_of_t[et]
    key = (ec * NG + eg) * NQ + eq
    order = np.lexsort((et, key))
    kcnt = np.bincount(key, minlength=NC * NG * NQ)
    kstart = np.concatenate([[0], np.cumsum(kcnt)])[:-1]
    ranks = np.empty(E, np.int64)
    ranks[order] = np.arange(E) - kstart[key[order]]

    flat = slot_start[eg, eq] + ranks

    qidx_slots = np.zeros((NC, S_total), np.int16)
    dstv_slots = np.full((NC, S_total), SENT, _f32)   # epos + 128*et
    qidx_slots[ec, flat] = eqidx
    dstv_slots[ec, flat] = (epos + P * et).astype(_f32)

    idx16 = np.empty((NC, 32, S_total // 16), np.int16)
    for c in range(NC):
        wrapped = qidx_slots[c].reshape(-1, 16).T
        idx16[c] = np.tile(wrapped, (2, 1))   # 32 rows; replicated to 128 on-chip

    # incidences: (tile, quarter, chunk) union over cores
    # per (c,g,q): tile edge ranges from cumulative counts
    inc_of_tile = [[] for _ in range(TPC)]   # t -> list of (q, jrel)
    for g, (ta, tb) in enumerate(groups):
        for q in range(NQ):
            nch = int(L[g, q]) // P
            if nch == 0:
                continue
            hit = np.zeros((tb - ta, nch), bool)
            for c in range(NC):
                starts = np.concatenate(
                    [[0], np.cumsum(cnt[c, ta:tb, q])]
                )  # [ntiles+1]
                for ti in range(tb - ta):
                    s, e = starts[ti], starts[ti + 1]
                    if e <= s:
                        continue
                    j0, j1 = s // P, (e - 1) // P
                    hit[ti, j0 : j1 + 1] = True
            for ti in range(tb - ta):
                for j in range(nch):
                    if hit[ti, j]:
                        inc_of_tile[ta + ti].append((q, j))

    # tile-major incidence columns
    col_start = np.zeros(TPC + 1, np.int64)
    for t in range(TPC):
        col_start[t + 1] = col_start[t] + len(inc_of_tile[t])
    NINC = int(col_start[-1])

    dstl = np.empty((NC, P, NINC), _f32)
    for t in range(TPC):
        g = gid_of_t[t]
        for k, (q, j) in enumerate(inc_of_tile[t]):
            s0 = slot_start[g, q] + j * P
            col = dstv_slots[:, s0 : s0 + P] - P * t   # [NC, 128]
            dstl[:, :, col_start[t] + k] = col
    dstl = dstl.astype(BF16)

    return {
        "L": L,
        "slot_start": slot_start,
        "S_total": S_total,
        "groups": groups,
        "inc_of_tile": inc_of_tile,
        "col_start": col_start,
        "idx16": idx16,
        "dstl": dstl,
        "node_of": node_of,
        "xtab_order": xtab_order,
    }


def _build_program(L, slot_start, S_total, groups, inc_of_tile, col_start,
                   has_bias, repeat=1):
    import concourse.bacc as bacc
    import concourse.mybir as mybir
    import concourse.tile as tile
    from contextlib import ExitStack

    f32 = mybir.dt.float32
    bf = mybir.dt.bfloat16
    nc = bacc.Bacc(
        "TRN2", target_bir_lowering=False, debug=False, num_devices=NC,
        num_swdge_queues=4,
    )

    NINC = int(col_start[-1])
    x_d = nc.dram_tensor("x", [N, D], bf, kind="ExternalInput")
    xt_d = nc.dram_tensor("xt", [SLOTS, D], bf, kind="ExternalInput")
    idx_d = nc.dram_tensor("idx16", [32, S_total // 16], mybir.dt.int16, kind="ExternalInput")
    dst_d = nc.dram_tensor("dstl", [P, NINC], bf, kind="ExternalInput")
    wt_d = nc.dram_tensor("wt", [D, D], bf, kind="ExternalInput")
    b_d = nc.dram_tensor("bias", [1, D], bf, kind="ExternalInput")
    out_d = nc.dram_tensor("out", [SLOTS, D], bf, kind="ExternalOutput")

    with tile.TileContext(nc) as tc, ExitStack() as ctx:
        const = ctx.enter_context(tc.tile_pool(name="const", bufs=1))
        gxp = ctx.enter_context(tc.tile_pool(name="gx", bufs=12))
        ohp = ctx.enter_context(tc.tile_pool(name="oh", bufs=4))
        xtp = ctx.enter_context(tc.tile_pool(name="xt", bufs=4))
        htp = ctx.enter_context(tc.tile_pool(name="ht", bufs=4))
        obp = ctx.enter_context(tc.tile_pool(name="ob", bufs=4))
        pag = ctx.enter_context(tc.tile_pool(name="pagg", bufs=4, space="PSUM"))
        pou = ctx.enter_context(tc.tile_pool(name="pout", bufs=2, space="PSUM"))

        idxs_s = const.tile([32, S_total // 16], mybir.dt.int16)
        nc.sync.dma_start(out=idxs_s[:], in_=idx_d[:])
        idx_t = const.tile([P, S_total // 16], mybir.dt.int16)
        for _k in range(4):
            nc.vector.tensor_copy(
                out=idx_t[_k * 32 : (_k + 1) * 32, :], in_=idxs_s[:]
            )
        dst_t = const.tile([P, NINC], bf)
        nc.sync.dma_start(out=dst_t[:], in_=dst_d[:])
        wt_t = const.tile([D, D], bf)
        nc.sync.dma_start(out=wt_t[:], in_=wt_d[:])
        b_t = const.tile([1, D], bf)
        nc.sync.dma_start(out=b_t[:], in_=b_d[:])
        ones_t = const.tile([1, D], bf)
        nc.vector.memset(ones_t[:], 1.0)
        iota_i = const.tile([P, P], mybir.dt.int32)
        nc.gpsimd.iota(iota_i[:], pattern=[[1, P]], base=0, channel_multiplier=0)
        iota_b = const.tile([P, P], bf)
        nc.vector.tensor_copy(out=iota_b[:], in_=iota_i[:])

        for _rep in range(repeat):
            for g, (ta, tb) in enumerate(groups):
                gx_tiles = {}
                for q in range(NQ):
                    Lgq = int(L[g, q])
                    if Lgq == 0:
                        continue
                    c0 = int(slot_start[g, q]) // P
                    gx = gxp.tile([P, Lgq // P, P], bf, tag="gx")
                    nc.gpsimd.dma_gather(
                        gx[:],
                        x_d[q * QROWS : (q + 1) * QROWS, :],
                        idx_t[:, c0 * 8 : c0 * 8 + Lgq // 16],
                        Lgq,
                        Lgq,
                        D,
                        queue_num=q,
                    )
                    gx_tiles[q] = gx
                for t in range(ta, tb):
                    incs = inc_of_tile[t]
                    nch = len(incs)
                    xt_t = xtp.tile([P, P], bf, tag="xt")
                    nc.sync.dma_start(out=xt_t[:], in_=xt_d[t * P : (t + 1) * P, :])
                    psum = pag.tile([P, P], f32, space="PSUM", tag="pagg")
                    if nch:
                        cst = int(col_start[t])
                        oh = ohp.tile([P, nch, P], bf, tag="oh")
                        nc.vector.tensor_tensor(
                            out=oh[:],
                            in0=iota_b[:].unsqueeze(1).to_broadcast([P, nch, P]),
                            in1=dst_t[:, cst : cst + nch].unsqueeze(2).to_broadcast([P, nch, P]),
                            op=mybir.AluOpType.is_equal,
                        )
                        for i, (q, j) in enumerate(incs):
                            nc.tensor.matmul(
                                out=psum[:],
                                lhsT=gx_tiles[q][:, j, :],
                                rhs=oh[:, i, :],
                                start=(i == 0),
                                stop=(i == nch - 1),
                            )
                    ht = htp.tile([P, P], bf, tag="ht")
                    if nch:
                        nc.vector.tensor_tensor(
                            out=ht[:], in0=psum[:], in1=xt_t[:],
                            op=mybir.AluOpType.add,
                        )
                    else:
                        nc.vector.tensor_copy(out=ht[:], in_=xt_t[:])
                    po = pou.tile([P, P], f32, space="PSUM", tag="pout")
                    if has_bias:
                        nc.tensor.matmul(out=po[:], lhsT=ht[:], rhs=wt_t[:], start=True, stop=False)
                        nc.tensor.matmul(out=po[:], lhsT=ones_t[:], rhs=b_t[:], start=False, stop=True)
                    else:
                        nc.tensor.matmul(out=po[:], lhsT=ht[:], rhs=wt_t[:], start=True, stop=True)
                    ob = obp.tile([P, P], bf, tag="ob")
                    nc.scalar.activation(ob[:], po[:], mybir.ActivationFunctionType.Relu)
                    nc.sync.dma_start(out=out_d[t * P : (t + 1) * P, :], in_=ob[:])
    nc.compile()
    return nc


def _prepare(x, edge_index, W, b, repeat=1):
    x = np.ascontiguousarray(np.asarray(x, dtype=_f32))
    W = np.asarray(W, dtype=_f32)
    b = np.asarray(b, dtype=_f32)
    pre = _preprocess(edge_index)
    has_bias = bool(np.any(b))
    nc = _build_program(
        pre["L"], pre["slot_start"], pre["S_total"], pre["groups"],
        pre["inc_of_tile"], pre["col_start"], has_bias, repeat=repeat,
    )
    xb = x.astype(BF16)
    xtab = np.ascontiguousarray(xb[pre["xtab_order"]])
    wt = np.ascontiguousarray(W.T).astype(BF16)
    brow = np.ascontiguousarray(b.reshape(1, D)).astype(BF16)
    node_of = pre["node_of"]
    in_maps = []
    for c in range(NC):
        nidx = np.where(node_of[c] < 0, 0, node_of[c])
        xt = np.ascontiguousarray(
            xb[nidx].reshape(TPC, P, D).transpose(0, 2, 1).reshape(SLOTS, D)
        )
        in_maps.append(
            {
                "x": xtab,
                "xt": xt,
                "idx16": np.ascontiguousarray(pre["idx16"][c]),
                "dstl": np.ascontiguousarray(pre["dstl"][c]),
                "wt": wt,
                "bias": brow,
            }
        )
    return nc, in_maps, node_of


def _assemble(results, node_of):
    out = np.empty((N, D), _f32)
    for c in range(NC):
        oc = np.asarray(results[c]["out"], dtype=_f32)
        m = node_of[c] >= 0
        out[node_of[c][m]] = oc[m]
    return out


def kernel(x, edge_index, W, b):
    from concourse.bass_utils import run_bass_kernel_spmd

    nc, in_maps, node_of = _prepare(x, edge_index, W, b)
    res = run_bass_kernel_spmd(nc, in_maps, core_ids=list(range(NC)))
    return _assemble(res.results, node_of)
